# revision 1
# baseline (speedup 1.0000x reference)
"""Trainium2 Bass kernel for batched masked Kabsch-RMSD (Coords2RMSD loss).

Problem: for each of 4096 samples (1024 max atoms, variable num_atoms),
compute RMSD after optimal rigid alignment (Kabsch). Data-parallel over
8 NeuronCores (512 samples each), samples on SBUF partitions.

Math (per sample, avoids explicit centering):
  mask_i = i < n;  xm = mask*x, ym = mask*y   (interleaved [1024,3] coords)
  Sx_j = sum_i xm_ij, Sy likewise; sxx = sum xm^2, syy = sum ym^2
  R_jk = sum_i xm_ij ym_ik
  Rc = R - Sx Sy^T / n;  ex = sxx - |Sx|^2/n;  ey = syy - |Sy|^2/n
  M = Rc^T Rc;  eigenvalues via Smith's closed form (acos/cos through
  the ScalarE Arctan/Sin tables);  d = sign(det Rc)
  s = sqrt(l1)+sqrt(l2)+d*sqrt(l3);  rmsd = sqrt(max((ex+ey-2s)/n, 1e-12))

Engine split per 128-sample tile:
  DVE : mask compare, 6 fused mask-apply+centroid-sum STTs (strided fp32
        reads, dense de-interleaved writes), 5 fused product+accum STTs
  GPS : 4 fused product+accum STTs (dense reads)
  ACT : 2 Square+accum passes for the norms
Covariance products use scalar_tensor_tensor's accum_out so no separate
reduction passes exist anywhere.
"""

import math
import numpy as np

import concourse.bass as bass
import concourse.mybir as mybir
from concourse.bass_utils import run_bass_kernel_spmd
from concourse.tile import TileContext

F32 = mybir.dt.float32
BF16 = mybir.dt.bfloat16
I32 = mybir.dt.int32
ALU = mybir.AluOpType
ACT = mybir.ActivationFunctionType

N_CORES = 8
B_FULL = 4096
B_CORE = B_FULL // N_CORES        # 512
N_ATOMS = 1024
ROW = 3 * N_ATOMS                 # 3072
N_TILES = B_CORE // 128           # 4

# products assigned to gpsimd (dense reads only; rest go to DVE)
GPS_PRODUCTS = 6
GPS_MASKS = 2
BF16_PRODUCTS = True


def _build_kernel(split_waits: bool = True) -> bass.Bass:
    nc = bass.Bass()

    inp = nc.declare_dram_parameter("input", [B_CORE, ROW], F32, isOutput=False)
    tgt = nc.declare_dram_parameter("target", [B_CORE, ROW], F32, isOutput=False)
    nat = nc.declare_dram_parameter("num_atoms", [B_CORE], I32, isOutput=False)
    out = nc.declare_dram_parameter("out", [B_CORE], F32, isOutput=True)

    with TileContext(nc) as tc:
        with tc.tile_pool(name="p", bufs=1) as pool, \
             tc.tile_pool(name="io", bufs=2) as iop:

            # ---------- one-time setup ----------
            iota_i = pool.tile([128, N_ATOMS], I32, tag="iota_i")
            nc.gpsimd.iota(iota_i[:], pattern=[[1, N_ATOMS]], base=0,
                           channel_multiplier=0)
            iota_f = pool.tile([128, N_ATOMS], F32, tag="iota_f")
            nc.vector.tensor_copy(iota_f[:], iota_i[:])

            # num_atoms -> [128, N_TILES] i32 -> f32, inv_n
            n_i = pool.tile([128, N_TILES], I32, tag="n_i")
            nc.sync.dma_start(out=n_i[:],
                              in_=nat[:].rearrange("(t p) -> p t", p=128))
            nf = pool.tile([128, N_TILES], F32, tag="nf")
            nc.vector.tensor_copy(nf[:], n_i[:])
            inv_n = pool.tile([128, N_TILES], F32, tag="inv_n")
            nc.vector.reciprocal(inv_n[:], nf[:])

            # ---------- per-core stats ----------
            R9 = pool.tile([128, N_TILES, 9], F32, tag="R9")
            Sx = pool.tile([128, N_TILES, 3], F32, tag="Sx")
            Sy = pool.tile([128, N_TILES, 3], F32, tag="Sy")
            sxx = pool.tile([128, N_TILES], F32, tag="sxx")
            syy = pool.tile([128, N_TILES], F32, tag="syy")

            SDT = BF16 if BF16_PRODUCTS else F32
            act_scr = pool.tile([128, ROW], SDT, tag="act_scr")
            dve_scr = pool.tile([128, N_ATOMS], SDT, tag="dve_scr")

            # ---------- main loop over 4 sample tiles ----------
            for t in range(N_TILES):
                x = iop.tile([128, ROW], F32, tag="x")
                y = iop.tile([128, ROW], F32, tag="y")
                nc.sync.dma_start(out=x[:], in_=inp[t * 128 : (t + 1) * 128, :])
                nc.sync.dma_start(out=y[:], in_=tgt[t * 128 : (t + 1) * 128, :])

                mask = iop.tile([128, N_ATOMS], F32, tag="mask")
                nc.vector.tensor_scalar(mask[:], iota_f[:], nf[:, t : t + 1],
                                        None, ALU.is_lt)

                # de-interleaved masked coords, dense [128, 3, 1024]
                MDT = BF16 if BF16_PRODUCTS else F32
                xm = iop.tile([128, 3, N_ATOMS], MDT, tag="xm")
                ym = iop.tile([128, 3, N_ATOMS], MDT, tag="ym")
                xs = x[:].rearrange("p (n c) -> p c n", c=3)
                ys = y[:].rearrange("p (n c) -> p c n", c=3)
                # (tensor, coord) mask-apply jobs; last GPS_MASKS go to gpsimd
                # as plain TT (their centroid sums via cheap DVE TS+accum).
                jobs = [(xm, xs, Sx, c) for c in range(3)] + \
                       [(ym, ys, Sy, c) for c in range(3)]
                for ji, (dst, srcv, Ssum, c) in enumerate(jobs):
                    acc = Ssum[:, t, c : c + 1]
                    if ji >= len(jobs) - GPS_MASKS:
                        nc.gpsimd.tensor_tensor(dst[:, c, :], srcv[:, c, :],
                                                mask[:], ALU.mult)
                        nc.vector.tensor_scalar(dve_scr[:], dst[:, c, :], 1.0,
                                                0.0, ALU.mult, ALU.add,
                                                accum_out=acc)
                    else:
                        nc.vector.scalar_tensor_tensor(
                            dst[:, c, :], srcv[:, c, :], 1.0, mask[:],
                            ALU.mult, ALU.mult, accum_out=acc)

                # norms on ScalarE (Square + accumulate over the whole row)
                nc.scalar.activation(act_scr[:], xm[:].rearrange("p c n -> p (c n)"),
                                     ACT.Square, accum_out=sxx[:, t : t + 1])
                nc.scalar.activation(act_scr[:], ym[:].rearrange("p c n -> p (c n)"),
                                     ACT.Square, accum_out=syy[:, t : t + 1])

                # 9 covariance products. walrus only lowers the fused
                # STT(+accum) on DVE, so gpsimd gets plain tensor_tensor
                # into scratch and ScalarE accumulates those afterwards.
                gps_scr = iop.tile([128, GPS_PRODUCTS, N_ATOMS], SDT, tag="gps_scr")
                pairs = [(j, k) for j in range(3) for k in range(3)]
                for idx, (j, k) in enumerate(pairs):
                    r9col = R9[:, t, 3 * j + k : 3 * j + k + 1]
                    if idx < GPS_PRODUCTS:
                        nc.gpsimd.tensor_tensor(gps_scr[:, idx, :], xm[:, j, :],
                                                ym[:, k, :], ALU.mult)
                        if idx % 2 == 0:
                            # accumulate on DVE (4x bf16 tensor_scalar)
                            nc.vector.tensor_scalar(
                                dve_scr[:], gps_scr[:, idx, :], 1.0, 0.0,
                                ALU.mult, ALU.add, accum_out=r9col)
                        else:
                            nc.scalar.activation(act_scr[:, :N_ATOMS],
                                                 gps_scr[:, idx, :],
                                                 ACT.Copy, accum_out=r9col)
                    else:
                        nc.vector.scalar_tensor_tensor(
                            dve_scr[:], xm[:, j, :], 1.0, ym[:, k, :],
                            ALU.mult, ALU.mult, accum_out=r9col)

            # ---------- batched eigensolve / RMSD ([128, N_TILES]) ----------
            T = N_TILES

            def tile4(shape, tag):
                return pool.tile(shape, F32, tag=tag, name=tag)

            v = nc.vector
            s_ = nc.scalar


            # Rc = R9 - (Sx outer Sy) * inv_n
            Rc = tile4([128, T, 9], "Rc")
            t9a = tile4([128, T, 9], "t9a")
            sxb = Sx[:].broadcast_to((128, T, 3, 3))
            syb = Sy[:].rearrange("p t k -> p t () k").broadcast_to((128, T, 3, 3))
            v.tensor_tensor(t9a[:].rearrange("p t (j k) -> p t j k", k=3),
                            sxb, syb, ALU.mult)
            invb9 = inv_n[:].rearrange("p t -> p t ()").broadcast_to((128, T, 9))
            v.tensor_tensor(t9a[:], t9a[:], invb9, ALU.mult)
            v.tensor_tensor(Rc[:], R9[:], t9a[:], ALU.subtract)

            # ex = sxx - |Sx|^2 * inv_n ; ey likewise
            t3 = tile4([128, T, 3], "t3")
            tA = tile4([128, T], "tA")
            tB = tile4([128, T], "tB")
            ex = tile4([128, T], "ex")
            ey = tile4([128, T], "ey")
            v.tensor_tensor(t3[:], Sx[:], Sx[:], ALU.mult)
            v.tensor_reduce(tA[:], t3[:], mybir.AxisListType.X, ALU.add)
            v.tensor_tensor(tA[:], tA[:], inv_n[:], ALU.mult)
            v.tensor_tensor(ex[:], sxx[:], tA[:], ALU.subtract)
            v.tensor_tensor(t3[:], Sy[:], Sy[:], ALU.mult)
            v.tensor_reduce(tB[:], t3[:], mybir.AxisListType.X, ALU.add)
            v.tensor_tensor(tB[:], tB[:], inv_n[:], ALU.mult)
            v.tensor_tensor(ey[:], syy[:], tB[:], ALU.subtract)

            def col(ap, i):            # [128, T] column i of a [128,T,9] tile
                return ap[:, :, i]

            r00, r01, r02 = (col(Rc, i) for i in range(3))
            r10, r11, r12 = (col(Rc, i) for i in range(3, 6))
            r20, r21, r22 = (col(Rc, i) for i in range(6, 9))

            # det(Rc) via cofactors
            c0 = tile4([128, T], "c0")
            c1 = tile4([128, T], "c1")
            c2 = tile4([128, T], "c2")
            u0 = tile4([128, T], "u0")
            u1 = tile4([128, T], "u1")
            det = tile4([128, T], "det")

            def msub(dst, a, b, c, d):     # dst = a*b - c*d
                v.tensor_tensor(u0[:], a, b, ALU.mult)
                v.tensor_tensor(u1[:], c, d, ALU.mult)
                v.tensor_tensor(dst, u0[:], u1[:], ALU.subtract)

            msub(c0[:], r11, r22, r12, r21)
            msub(c1[:], r10, r22, r12, r20)
            msub(c2[:], r10, r21, r11, r20)
            v.tensor_tensor(c0[:], c0[:], r00, ALU.mult)
            v.tensor_tensor(c1[:], c1[:], r01, ALU.mult)
            v.tensor_tensor(c2[:], c2[:], r02, ALU.mult)
            v.tensor_tensor(det[:], c0[:], c1[:], ALU.subtract)
            v.tensor_tensor(det[:], det[:], c2[:], ALU.add)

            # M = Rc^T Rc (6 unique entries)
            M6 = tile4([128, T, 6], "M6")      # M00 M11 M22 M01 M02 M12
            Rcv = Rc[:].rearrange("p t (j k) -> p t j k", k=3)
            mpairs = [(0, 0), (1, 1), (2, 2), (0, 1), (0, 2), (1, 2)]
            for i, (a, b) in enumerate(mpairs):
                v.tensor_tensor(t3[:], Rcv[:, :, :, a], Rcv[:, :, :, b], ALU.mult)
                v.tensor_reduce(M6[:, :, i], t3[:], mybir.AxisListType.X, ALU.add)

            M00, M11, M22 = (M6[:, :, i] for i in range(3))
            M01, M02, M12 = (M6[:, :, i] for i in range(3, 6))

            # q = tr/3 ; p = sqrt(p2/6) with p2 = sum aii^2 + 2*(off^2)
            q = tile4([128, T], "q")
            v.tensor_tensor(q[:], M00, M11, ALU.add)
            v.tensor_tensor(q[:], q[:], M22, ALU.add)
            v.tensor_scalar(q[:], q[:], 1.0 / 3.0, None, ALU.mult)

            a00 = tile4([128, T], "a00")
            a11 = tile4([128, T], "a11")
            a22 = tile4([128, T], "a22")
            v.tensor_tensor(a00[:], M00, q[:], ALU.subtract)
            v.tensor_tensor(a11[:], M11, q[:], ALU.subtract)
            v.tensor_tensor(a22[:], M22, q[:], ALU.subtract)

            p2 = tile4([128, T], "p2")
            v.tensor_tensor(u0[:], M01, M01, ALU.mult)
            v.tensor_tensor(u1[:], M02, M02, ALU.mult)
            v.tensor_tensor(p2[:], u0[:], u1[:], ALU.add)
            v.tensor_tensor(u0[:], M12, M12, ALU.mult)
            v.tensor_tensor(p2[:], p2[:], u0[:], ALU.add)
            v.tensor_scalar(p2[:], p2[:], 2.0, None, ALU.mult)
            v.tensor_tensor(u0[:], a00[:], a00[:], ALU.mult)
            v.tensor_tensor(p2[:], p2[:], u0[:], ALU.add)
            v.tensor_tensor(u0[:], a11[:], a11[:], ALU.mult)
            v.tensor_tensor(p2[:], p2[:], u0[:], ALU.add)
            v.tensor_tensor(u0[:], a22[:], a22[:], ALU.mult)
            v.tensor_tensor(p2[:], p2[:], u0[:], ALU.add)

            pp = tile4([128, T], "pp")
            v.tensor_scalar(pp[:], p2[:], 1.0 / 6.0, None, ALU.mult)
            s_.activation(pp[:], pp[:], ACT.Sqrt)
            ip = tile4([128, T], "ip")
            v.tensor_scalar(u0[:], pp[:], 1e-20, None, ALU.max)
            v.reciprocal(ip[:], u0[:])

            # detB = det(M - qI) ; r = detB * ip^3 / 2, clamped to [-1, 1]
            detB = tile4([128, T], "detB")
            msub(c0[:], a11[:], a22[:], M12, M12)
            msub(c1[:], M01, a22[:], M12, M02)
            msub(c2[:], M01, M12, a11[:], M02)
            v.tensor_tensor(c0[:], c0[:], a00[:], ALU.mult)
            v.tensor_tensor(c1[:], c1[:], M01, ALU.mult)
            v.tensor_tensor(c2[:], c2[:], M02, ALU.mult)
            v.tensor_tensor(detB[:], c0[:], c1[:], ALU.subtract)
            v.tensor_tensor(detB[:], detB[:], c2[:], ALU.add)

            rr = tile4([128, T], "rr")
            v.tensor_tensor(u0[:], ip[:], ip[:], ALU.mult)
            v.tensor_tensor(u0[:], u0[:], ip[:], ALU.mult)
            v.tensor_tensor(rr[:], detB[:], u0[:], ALU.mult)
            v.tensor_scalar(rr[:], rr[:], 0.5, None, ALU.mult)
            v.tensor_scalar(rr[:], rr[:], 1.0, -1.0, ALU.min, ALU.max)

            # c = cos(acos(r)/3) is the root of 4c^3-3c=r in [1/2,1].
            # Newton (table-free, avoids ACT Arctan/Sin set loads):
            #   c <- (8c^3 + r) / (12c^2 - 3), from c=1, 7 iterations.
            # Errors at the degenerate r=-1 corner are benign: lam1+lam2 is
            # trace-compensated and d(t3)/dc = 0 at c=1/2.
            cc = tile4([128, T], "cc")
            u2 = tile4([128, T], "u2")
            # init: quadratic fit of cos(acos(r)/3), max err ~5e-3
            v.tensor_scalar(cc[:], rr[:], -0.116, 0.25, ALU.mult, ALU.add)
            v.scalar_tensor_tensor(cc[:], rr[:], 1.0, cc[:], ALU.mult, ALU.mult)
            v.tensor_scalar(cc[:], cc[:], 1.0, 0.866, ALU.mult, ALU.add)
            for _ in range(3):
                v.tensor_tensor(u0[:], cc[:], cc[:], ALU.mult)          # c^2
                v.tensor_scalar(u2[:], u0[:], 12.0, -3.0, ALU.mult, ALU.add)
                v.tensor_tensor(u0[:], u0[:], cc[:], ALU.mult)          # c^3
                v.scalar_tensor_tensor(u0[:], u0[:], 8.0, rr[:],
                                       ALU.mult, ALU.add)               # 8c^3+r
                v.reciprocal(u2[:], u2[:])
                v.tensor_tensor(cc[:], u0[:], u2[:], ALU.mult)
            sphi = tile4([128, T], "sphi")
            v.tensor_tensor(u0[:], cc[:], cc[:], ALU.mult)
            v.tensor_scalar(u0[:], u0[:], -1.0, 1.0, ALU.mult, ALU.add)  # 1-c^2
            v.tensor_scalar(u0[:], u0[:], 0.0, None, ALU.max)
            s_.activation(sphi[:], u0[:], ACT.Sqrt)

            # lam1 = q + 2p*c ; lam3 = q + 2p*(-c/2 - (sqrt3/2) sphi) ; lam2 = 3q-l1-l3
            lam1 = tile4([128, T], "lam1")
            lam2 = tile4([128, T], "lam2")
            lam3 = tile4([128, T], "lam3")
            v.tensor_tensor(u0[:], pp[:], cc[:], ALU.mult)
            v.tensor_scalar(u0[:], u0[:], 2.0, None, ALU.mult)
            v.tensor_tensor(lam1[:], q[:], u0[:], ALU.add)

            v.tensor_scalar(u0[:], cc[:], -0.5, None, ALU.mult)
            v.scalar_tensor_tensor(u0[:], sphi[:], -math.sqrt(3.0) / 2.0, u0[:],
                                   ALU.mult, ALU.add)
            v.tensor_tensor(u0[:], u0[:], pp[:], ALU.mult)
            v.tensor_scalar(u0[:], u0[:], 2.0, None, ALU.mult)
            v.tensor_tensor(lam3[:], q[:], u0[:], ALU.add)

            v.tensor_scalar(u0[:], q[:], 3.0, None, ALU.mult)
            v.tensor_tensor(lam2[:], u0[:], lam1[:], ALU.subtract)
            v.tensor_tensor(lam2[:], lam2[:], lam3[:], ALU.subtract)

            # s = sqrt(l1) + sqrt(l2) + sign(det)*sqrt(l3)
            for lam in (lam1, lam2, lam3):
                v.tensor_scalar(lam[:], lam[:], 0.0, None, ALU.max)
                s_.activation(lam[:], lam[:], ACT.Sqrt)

            dsign = tile4([128, T], "dsign")
            v.tensor_scalar(dsign[:], det[:], 0.0, None, ALU.is_ge)
            v.tensor_scalar(dsign[:], dsign[:], 2.0, -1.0, ALU.mult, ALU.add)

            ssum = tile4([128, T], "ssum")
            v.tensor_tensor(ssum[:], lam1[:], lam2[:], ALU.add)
            v.tensor_tensor(u0[:], dsign[:], lam3[:], ALU.mult)
            v.tensor_tensor(ssum[:], ssum[:], u0[:], ALU.add)

            # rmsd = sqrt(max((ex + ey - 2 s) / n, 1e-12))
            res = tile4([128, T], "res")
            v.tensor_tensor(res[:], ex[:], ey[:], ALU.add)
            v.scalar_tensor_tensor(res[:], ssum[:], -2.0, res[:], ALU.mult, ALU.add)
            v.tensor_tensor(res[:], res[:], inv_n[:], ALU.mult)
            v.tensor_scalar(res[:], res[:], 1e-12, None, ALU.max)
            s_.activation(res[:], res[:], ACT.Sqrt)

            nc.sync.dma_start(
                out=out[:].rearrange("(t p) -> p t", p=128),
                in_=res[:])

    if split_waits:
        _split_multi_waits(nc)
    return nc


def _split_multi_waits(nc):
    """walrus rejects >1 sync-wait on DVE instruction structs; move extra
    waits onto single-wait NoOp carriers queued just before, same engine."""
    ctr = 0
    for f in nc.m.functions:
        for bb in f.blocks:
            new = []
            for inst in bb.instructions:
                si = inst.sync_info
                if si is not None and si.on_wait and len(si.on_wait) > 1:
                    waits = list(si.on_wait)
                    for w in waits[:-1]:
                        ctr += 1
                        new.append(mybir.InstNoOp(
                            name=f"waitnop-{ctr}", engine=inst.engine,
                            ins=[], outs=[],
                            sync_info=mybir.SyncInfo(on_wait=[w],
                                                     on_update=[])))
                    inst.sync_info = mybir.SyncInfo(on_wait=[waits[-1]],
                                                    on_update=si.on_update)
                new.append(inst)
            bb.instructions = new


_NC_CACHE = None


def _get_nc():
    global _NC_CACHE
    if _NC_CACHE is None:
        _NC_CACHE = _build_kernel()
    return _NC_CACHE


def kernel(input: np.ndarray, target: np.ndarray, num_atoms: np.ndarray,
           **_unused) -> np.ndarray:
    input = np.ascontiguousarray(np.asarray(input, dtype=np.float32))
    target = np.ascontiguousarray(np.asarray(target, dtype=np.float32))
    num_atoms = np.ascontiguousarray(np.asarray(num_atoms, dtype=np.int32))

    nc = _get_nc()
    in_maps = []
    for i in range(N_CORES):
        sl = slice(i * B_CORE, (i + 1) * B_CORE)
        in_maps.append({
            "input": input[sl],
            "target": target[sl],
            "num_atoms": num_atoms[sl],
        })
    res = run_bass_kernel_spmd(nc, in_maps, list(range(N_CORES)))
    outs = [res.results[i]["out"].reshape(B_CORE) for i in range(N_CORES)]
    return np.concatenate(outs).astype(np.float32)


if __name__ == "__main__":
    rng = np.random.default_rng(0)
    inp = rng.standard_normal((B_FULL, ROW), dtype=np.float32)
    tgt = rng.standard_normal((B_FULL, ROW), dtype=np.float32)
    na = rng.integers(8, N_ATOMS + 1, size=(B_FULL,), dtype=np.int32)
    print(kernel(input=inp, target=tgt, num_atoms=na)[:8])

